# revision 16
# baseline (speedup 1.0000x reference)
"""Trainium2 Bass kernel for nn_Attention_51410758533700.

Computes, for q,k,v [b=2, h=16, n=2048, d=64] f32:
  q' = rope(l2norm(q) * q_scale), k' = rope(l2norm(k) * k_scale)
  out = softmax(q' k'^T / sqrt(d)) @ v, returned as [b, n, h*d].

Sharding: the 32 (b,h) pairs are split 4-per-core across 8 NeuronCores
(data+head parallel, no cross-core comms). Each core runs an identical
Bass/Tile program on its own slice.

Device algorithm (per pair, n=2048, d=64):
  - l2norm folded as: rope(x)*rsqrt(sumsq(x)); all rsqrt's for the whole
    kernel are computed in one batched Ln + one Exp (single activation
    table-set load each, no per-pair table thrash).
  - rope via host-precomputed cos/sin tables (scale-folded, sin table
    pre-shuffled/negated so rotate_half becomes a shifted AP).
  - qT/kT produced by PE transposes (f32), cast to bf16 on the PSUM->SBUF
    copy; S^T tiles [128j, 512i] via bf16 PE matmul (fp32 matmul runs
    2-pass LOW_HIGH at half throughput, so bf16 is 2x).
  - softmax without max-subtraction (|scores| <= 1/8 since q,k unit-norm);
    exp on ScalarE, scale=1/sqrt(d) folded in, output cast to bf16.
  - O^T = V'^T E^T accumulated in PSUM f32, V' carries a ones column so
    the softmax denominator falls out of the same matmul.
  - PE transpose of O^T back to [n, d], reciprocal row-sum normalize.
"""

import os
import sys

sys.path.insert(0, "/opt/trn_rl_repo")

import numpy as np

B, H, N, D = 2, 16, 2048, 64
N_CORES = 8
PAIRS = B * H
PPC = PAIRS // N_CORES  # pairs per core
NT = N // 128           # 16 n-tiles of 128
IC = 4                  # i-chunks per pair
ICW = N // IC           # 512

_CACHE = {}
LAST_RESULTS = None


def _rope_tables(q_scale, k_scale):
    """cos/sin tables with the per-dim scale and rotate_half sign folded in.

    t1 = x * cos_tab;  t2[:, :32] = x[:, 32:] * sin_tab[:, :32]
                       t2[:, 32:] = x[:, :32] * sin_tab[:, 32:]
    rope(x * s) == t1 + t2 with the tables below.
    """
    half = D // 2
    inv_freq = (np.float32(10000.0) **
                (-(np.arange(0, D, 2, dtype=np.float32) / np.float32(D))))
    seq = np.arange(N, dtype=np.float32)
    freqs = seq[:, None] * inv_freq[None, :]          # [N, 32]
    emb = np.concatenate([freqs, freqs], axis=1)      # [N, 64]
    cos = np.cos(emb).astype(np.float32)
    sin = np.sin(emb).astype(np.float32)

    def fold(scale):
        scale = scale.astype(np.float32)
        cos_t = cos * scale[None, :]
        sin_t = np.empty_like(sin)
        sin_t[:, :half] = -sin[:, :half] * scale[None, half:]
        sin_t[:, half:] = sin[:, half:] * scale[None, :half]
        return cos_t, sin_t

    qcos, qsin = fold(q_scale)
    kcos, ksin = fold(k_scale)
    return qcos, qsin, kcos, ksin


def _build():
    if "nc" in _CACHE:
        return _CACHE["nc"]

    from contextlib import ExitStack

    import concourse.bass as bass
    import concourse.tile as tile
    from concourse import bacc, mybir
    from concourse.masks import make_identity

    f32 = mybir.dt.float32
    bf16 = mybir.dt.bfloat16
    AF = mybir.ActivationFunctionType

    nc = bacc.Bacc("TRN2", target_bir_lowering=False, debug=False,
                   num_devices=N_CORES)

    q_t = nc.dram_tensor("q4", [PPC, N, D], f32, kind="ExternalInput")
    k_t = nc.dram_tensor("k4", [PPC, N, D], f32, kind="ExternalInput")
    v_t = nc.dram_tensor("v4", [PPC, N, D], f32, kind="ExternalInput")
    qcos_t = nc.dram_tensor("qcos", [N, D], f32, kind="ExternalInput")
    qsin_t = nc.dram_tensor("qsin", [N, D], f32, kind="ExternalInput")
    kcos_t = nc.dram_tensor("kcos", [N, D], f32, kind="ExternalInput")
    ksin_t = nc.dram_tensor("ksin", [N, D], f32, kind="ExternalInput")
    out_t = nc.dram_tensor("out4", [PPC, N, D], f32, kind="ExternalOutput")

    # n = t*128 + p  (tile t on the free axis, row p on the partition axis)
    qv = q_t.ap().rearrange("a (t p) d -> a p t d", p=128)
    kv = k_t.ap().rearrange("a (t p) d -> a p t d", p=128)
    vv = v_t.ap().rearrange("a (t p) d -> a p t d", p=128)
    outv = out_t.ap().rearrange("a (c t p) d -> a c p t d", c=IC, p=128)
    tabs = {
        "qcos": qcos_t.ap().rearrange("(t p) d -> p t d", p=128),
        "qsin": qsin_t.ap().rearrange("(t p) d -> p t d", p=128),
        "kcos": kcos_t.ap().rearrange("(t p) d -> p t d", p=128),
        "ksin": ksin_t.ap().rearrange("(t p) d -> p t d", p=128),
    }
    half = D // 2

    with tile.TileContext(nc) as tc, ExitStack() as ctx:
        consts = ctx.enter_context(tc.tile_pool(name="consts", bufs=1))
        ld = ctx.enter_context(tc.tile_pool(name="ld", bufs=2))
        prep = ctx.enter_context(tc.tile_pool(name="prep", bufs=2))
        small = ctx.enter_context(tc.tile_pool(name="small", bufs=4))
        pairp = ctx.enter_context(tc.tile_pool(name="pairp", bufs=2))
        epool = ctx.enter_context(tc.tile_pool(name="epool", bufs=2))
        opool = ctx.enter_context(tc.tile_pool(name="opool", bufs=2))
        spsum = ctx.enter_context(tc.tile_pool(name="spsum", bufs=1, space="PSUM"))
        tpsum = ctx.enter_context(tc.tile_pool(name="tpsum", bufs=2, space="PSUM"))

        identity = consts.tile([128, 128], f32)
        make_identity(nc, identity)
        identity_bf = consts.tile([128, 128], bf16)
        nc.vector.tensor_copy(out=identity_bf, in_=identity)
        zbias = consts.tile([128, 1], f32)
        nc.vector.memset(zbias, 0.0)
        tab_sb = {}
        for name, ap in tabs.items():
            t = consts.tile([128, NT, D], f32, tag=f"tab_{name}")
            nc.gpsimd.dma_start(out=t, in_=ap)
            tab_sb[name] = t

        def load_and_ssq(pr):
            """Load q,k; return raws and rsqrt(sumsq) [128, 32] (q|k) via
            DVE Newton (no activation-table traffic)."""
            raws = {}
            ssq2 = small.tile([128, 2 * NT], f32, tag="ssq2")
            for col, (which, view) in enumerate((("q", qv), ("k", kv))):
                raw = ld.tile([128, NT, D], f32, tag=f"raw_{which}")
                nc.sync.dma_start(out=raw, in_=view[pr])
                raws[which] = raw
                sq = prep.tile([128, NT, D], f32, tag="sq")
                nc.vector.tensor_mul(sq, raw, raw)
                nc.vector.tensor_reduce(
                    out=ssq2[:, col * NT:(col + 1) * NT], in_=sq,
                    axis=mybir.AxisListType.X, op=mybir.AluOpType.add)
            y = small.tile([128, 2 * NT], f32, tag="ny")
            nc.vector.memset(y, 0.12)
            t = small.tile([128, 2 * NT], f32, tag="nt")
            for _ in range(5):  # y <- y*(1.5 - 0.5*x*y^2)
                nc.vector.tensor_mul(t, y, y)
                nc.vector.tensor_mul(t, t, ssq2)
                nc.vector.tensor_scalar(out=t, in0=t, scalar1=-0.5,
                                        scalar2=1.5,
                                        op0=mybir.AluOpType.mult,
                                        op1=mybir.AluOpType.add)
                nc.vector.tensor_mul(y, y, t)
            return raws, y

        def rope_norm(raw, rinv, cos_sb, sin_sb, tag):
            """bf16 rope(raw)*rinv; rinv is a [128, NT] slice."""
            t1 = prep.tile([128, NT, D], f32, tag="t1")
            nc.vector.tensor_mul(t1, raw, cos_sb)
            t2 = prep.tile([128, NT, D], f32, tag="t2")
            nc.vector.tensor_mul(t2[:, :, 0:half], raw[:, :, half:D],
                                 sin_sb[:, :, 0:half])
            nc.vector.tensor_mul(t2[:, :, half:D], raw[:, :, 0:half],
                                 sin_sb[:, :, half:D])
            rope = prep.tile([128, NT, D], f32, tag="rope")
            nc.vector.tensor_add(rope, t1, t2)
            rb = bass.AP(tensor=rinv.tensor, offset=rinv.offset,
                         ap=[*rinv.ap, [0, D]])
            out = prep.tile([128, NT, D], bf16, tag=f"n_{tag}")
            nc.vector.tensor_mul(out, rope, rb)
            return out

        def transpose_to(dst_T, src):
            # src [128, NT, 64] bf16 -> dst_T [64, N] bf16 via PE transposes
            for g in range(NT // 4):
                ps = tpsum.tile([64, 4, 128], bf16, tag="tp")
                for u in range(4):
                    t = 4 * g + u
                    nc.tensor.transpose(out=ps[:, u, :], in_=src[:, t, :],
                                        identity=identity_bf)
                nc.vector.tensor_copy(
                    out=dst_T[:, g * 512:(g + 1) * 512].rearrange(
                        "p (a b) -> p a b", a=4),
                    in_=ps)

        def do_prep(pr):
            raws, rinv2 = load_and_ssq(pr)
            qn = rope_norm(raws["q"], rinv2[:, 0:NT],
                           tab_sb["qcos"], tab_sb["qsin"], "q")
            qT = pairp.tile([128, N], bf16, tag="qT")
            transpose_to(qT[0:64, :], qn)
            # duplicate into partitions 64:127 so row-packed matmuls
            # (tile_position row 64) can read lhsT/rhs from the upper half
            nc.sync.dma_start(out=qT[64:128, :], in_=qT[0:64, :])

            kn = rope_norm(raws["k"], rinv2[:, NT:2 * NT],
                           tab_sb["kcos"], tab_sb["ksin"], "k")
            kT = pairp.tile([128, N], bf16, tag="kT")
            transpose_to(kT[0:64, :], kn)
            nc.sync.dma_start(out=kT[64:128, :], in_=kT[0:64, :])

            vext = pairp.tile([128, NT, D + 1], bf16, tag="vext")
            nc.gpsimd.dma_start(out=vext[:, :, 0:D], in_=vv[pr])  # f32->bf16
            nc.vector.memset(vext[:, :, D:D + 1], 1.0)
            return qT, kT, vext

        def do_main(pr, qT, kT, vext):
            # j-tile groups per i-chunk, sized to alternate between a 4-bank
            # and a 2-bank PSUM score buffer so exp ops are as wide as the
            # ping-pong allows (3x 2048-elem + 2x 1024-elem per chunk)
            groups = [4, 2, 4, 2, 4]
            for ic in range(IC):
                esb = epool.tile([128, NT, ICW], bf16, tag="E")
                op = tpsum.tile([D + 1, ICW], f32, tag="tp")
                jt0 = 0
                for g, gsz in enumerate(groups):
                    sp = spsum.tile([128, gsz, ICW], f32,
                                    tag="SA" if gsz == 4 else "SB")
                    for u in range(gsz):
                        jt = jt0 + u
                        lo = 64 * (jt % 2)
                        nc.tensor.matmul(
                            out=sp[:, u, :],
                            lhsT=kT[lo:lo + 64, jt * 128:(jt + 1) * 128],
                            rhs=qT[lo:lo + 64, ic * ICW:(ic + 1) * ICW],
                            start=True, stop=True,
                            tile_position=(lo, 0))
                    nc.scalar.activation(out=esb[:, jt0:jt0 + gsz, :], in_=sp,
                                         func=AF.Exp, scale=float(D ** -0.5),
                                         bias=zbias)
                    for u in range(gsz):
                        jt = jt0 + u
                        nc.tensor.matmul(out=op, lhsT=vext[:, jt, :],
                                         rhs=esb[:, jt, :],
                                         start=(jt == 0), stop=(jt == NT - 1))
                    jt0 += gsz
                oc = opool.tile([D + 1, ICW], f32, tag="oc")
                nc.vector.tensor_copy(out=oc, in_=op)
                ot = tpsum.tile([128, 4, D + 1], f32, tag="tp")
                for tt in range(4):
                    nc.tensor.transpose(out=ot[:, tt, :],
                                        in_=oc[:, tt * 128:(tt + 1) * 128],
                                        identity=identity[0:D + 1, 0:D + 1])
                ro = small.tile([128, 4], f32, tag="ro")
                nc.vector.reciprocal(out=ro, in_=ot[:, :, D])
                osb = opool.tile([128, 4, D], f32, tag="osb")
                rob = bass.AP(tensor=ro.tensor, offset=ro.offset,
                              ap=[*ro.ap, [0, D]])
                nc.vector.tensor_mul(osb, ot[:, :, 0:D], rob)
                nc.sync.dma_start(out=outv[pr, ic], in_=osb)

        # software pipeline: pair p+1's prep is emitted (priority-ordered)
        # ahead of pair p's main loop so DVE prep overlaps PE/ACT main work
        handles = do_prep(0)
        for pr in range(PPC):
            nxt = do_prep(pr + 1) if pr + 1 < PPC else None
            do_main(pr, *handles)
            handles = nxt

    nc.compile()
    _CACHE["nc"] = nc
    return nc


def kernel(q, k, v, q_scale, k_scale):
    global LAST_RESULTS
    from concourse.bass_utils import run_bass_kernel_spmd

    nc = _build()
    q = np.ascontiguousarray(np.asarray(q, dtype=np.float32))
    k = np.ascontiguousarray(np.asarray(k, dtype=np.float32))
    v = np.ascontiguousarray(np.asarray(v, dtype=np.float32))
    qcos, qsin, kcos, ksin = _rope_tables(np.asarray(q_scale),
                                          np.asarray(k_scale))

    # pair index = b*H + h; core c owns pairs [c*PPC, (c+1)*PPC)
    qp = q.reshape(PAIRS, N, D)
    kp = k.reshape(PAIRS, N, D)
    vp = v.reshape(PAIRS, N, D)
    in_maps = []
    for c in range(N_CORES):
        sl = slice(c * PPC, (c + 1) * PPC)
        in_maps.append({
            "q4": qp[sl], "k4": kp[sl], "v4": vp[sl],
            "qcos": qcos, "qsin": qsin, "kcos": kcos, "ksin": ksin,
        })

    trace = bool(int(os.environ.get("KERNEL_TRACE", "0")))
    kwargs = {}
    if trace and os.environ.get("KERNEL_TRACE_DIR"):
        kwargs["tmpdir"] = os.environ["KERNEL_TRACE_DIR"]
    res = run_bass_kernel_spmd(nc, in_maps, list(range(N_CORES)),
                               trace=trace, **kwargs)
    LAST_RESULTS = res

    outp = np.concatenate([res.results[c]["out4"] for c in range(N_CORES)],
                          axis=0)                       # [32, N, D]
    out = outp.reshape(B, H, N, D).transpose(0, 2, 1, 3).reshape(B, N, H * D)
    return np.ascontiguousarray(out)


# revision 20
# speedup vs baseline: 1.2146x; 1.2146x over previous
"""Trainium2 Bass kernel for nn_Attention_51410758533700.

Computes, for q,k,v [b=2, h=16, n=2048, d=64] f32:
  q' = rope(l2norm(q) * q_scale), k' = rope(l2norm(k) * k_scale)
  out = softmax(q' k'^T / sqrt(d)) @ v, returned as [b, n, h*d].

Sharding: the 32 (b,h) pairs are split 4-per-core across 8 NeuronCores
(data+head parallel, no cross-core comms). Each core runs an identical
Bass/Tile program on its own slice.

Device algorithm (per pair, n=2048, d=64):
  - l2norm folded as: rope(x)*rsqrt(sumsq(x)); all rsqrt's for the whole
    kernel are computed in one batched Ln + one Exp (single activation
    table-set load each, no per-pair table thrash).
  - rope via host-precomputed cos/sin tables (scale-folded, sin table
    pre-shuffled/negated so rotate_half becomes a shifted AP).
  - qT/kT produced by PE transposes (f32), cast to bf16 on the PSUM->SBUF
    copy; S^T tiles [128j, 512i] via bf16 PE matmul (fp32 matmul runs
    2-pass LOW_HIGH at half throughput, so bf16 is 2x).
  - softmax without max-subtraction (|scores| <= 1/8 since q,k unit-norm);
    exp on ScalarE, scale=1/sqrt(d) folded in, output cast to bf16.
  - O^T = V'^T E^T accumulated in PSUM f32, V' carries a ones column so
    the softmax denominator falls out of the same matmul.
  - PE transpose of O^T back to [n, d], reciprocal row-sum normalize.
"""

import os
import sys

sys.path.insert(0, "/opt/trn_rl_repo")

import numpy as np

B, H, N, D = 2, 16, 2048, 64
N_CORES = 8
PAIRS = B * H
PPC = PAIRS // N_CORES  # pairs per core
NT = N // 128           # 16 n-tiles of 128
IC = 4                  # i-chunks per pair
ICW = N // IC           # 512

_CACHE = {}
LAST_RESULTS = None


def _rope_tables(q_scale, k_scale):
    """cos/sin tables with the per-dim scale and rotate_half sign folded in.

    t1 = x * cos_tab;  t2[:, :32] = x[:, 32:] * sin_tab[:, :32]
                       t2[:, 32:] = x[:, :32] * sin_tab[:, 32:]
    rope(x * s) == t1 + t2 with the tables below.
    """
    half = D // 2
    inv_freq = (np.float32(10000.0) **
                (-(np.arange(0, D, 2, dtype=np.float32) / np.float32(D))))
    seq = np.arange(N, dtype=np.float32)
    freqs = seq[:, None] * inv_freq[None, :]          # [N, 32]
    emb = np.concatenate([freqs, freqs], axis=1)      # [N, 64]
    cos = np.cos(emb).astype(np.float32)
    sin = np.sin(emb).astype(np.float32)

    def fold(scale):
        scale = scale.astype(np.float32)
        cos_t = cos * scale[None, :]
        sin_t = np.empty_like(sin)
        sin_t[:, :half] = -sin[:, :half] * scale[None, half:]
        sin_t[:, half:] = sin[:, half:] * scale[None, :half]
        return cos_t, sin_t

    qcos, qsin = fold(q_scale)
    kcos, ksin = fold(k_scale)
    return qcos, qsin, kcos, ksin


def _build():
    if "nc" in _CACHE:
        return _CACHE["nc"]

    from contextlib import ExitStack

    import concourse.bass as bass
    import concourse.tile as tile
    from concourse import bacc, mybir
    from concourse.masks import make_identity

    f32 = mybir.dt.float32
    bf16 = mybir.dt.bfloat16
    AF = mybir.ActivationFunctionType

    nc = bacc.Bacc("TRN2", target_bir_lowering=False, debug=False,
                   num_devices=N_CORES)

    q_t = nc.dram_tensor("q4", [PPC, N, D], f32, kind="ExternalInput")
    k_t = nc.dram_tensor("k4", [PPC, N, D], f32, kind="ExternalInput")
    v_t = nc.dram_tensor("v4", [PPC, N, D], f32, kind="ExternalInput")
    qcos_t = nc.dram_tensor("qcos", [N, D], f32, kind="ExternalInput")
    qsin_t = nc.dram_tensor("qsin", [N, D], f32, kind="ExternalInput")
    kcos_t = nc.dram_tensor("kcos", [N, D], f32, kind="ExternalInput")
    ksin_t = nc.dram_tensor("ksin", [N, D], f32, kind="ExternalInput")
    out_t = nc.dram_tensor("out4", [PPC, N, D], f32, kind="ExternalOutput")

    # n = t*128 + p  (tile t on the free axis, row p on the partition axis)
    qv = q_t.ap().rearrange("a (t p) d -> a p t d", p=128)
    kv = k_t.ap().rearrange("a (t p) d -> a p t d", p=128)
    vv = v_t.ap().rearrange("a (t p) d -> a p t d", p=128)
    outv = out_t.ap().rearrange("a (c t p) d -> a c p t d", c=IC, p=128)
    tabs = {
        "qcos": qcos_t.ap().rearrange("(t p) d -> p t d", p=128),
        "qsin": qsin_t.ap().rearrange("(t p) d -> p t d", p=128),
        "kcos": kcos_t.ap().rearrange("(t p) d -> p t d", p=128),
        "ksin": ksin_t.ap().rearrange("(t p) d -> p t d", p=128),
    }
    half = D // 2

    with tile.TileContext(nc) as tc, ExitStack() as ctx:
        consts = ctx.enter_context(tc.tile_pool(name="consts", bufs=1))
        ld = ctx.enter_context(tc.tile_pool(name="ld", bufs=2))
        prep = ctx.enter_context(tc.tile_pool(name="prep", bufs=2))
        small = ctx.enter_context(tc.tile_pool(name="small", bufs=4))
        pairp = ctx.enter_context(tc.tile_pool(name="pairp", bufs=2))
        epool = ctx.enter_context(tc.tile_pool(name="epool", bufs=2))
        opool = ctx.enter_context(tc.tile_pool(name="opool", bufs=2))
        spsum = ctx.enter_context(tc.tile_pool(name="spsum", bufs=2, space="PSUM"))
        opsum = ctx.enter_context(tc.tile_pool(name="opsum", bufs=2, space="PSUM"))
        tpsum = ctx.enter_context(tc.tile_pool(name="tpsum", bufs=2, space="PSUM"))

        identity = consts.tile([128, 128], f32)
        make_identity(nc, identity)
        identity_bf = consts.tile([128, 128], bf16)
        nc.vector.tensor_copy(out=identity_bf, in_=identity)
        zbias = consts.tile([128, 1], f32)
        nc.vector.memset(zbias, 0.0)
        tab_sb = {}
        for name, ap in tabs.items():
            t = consts.tile([128, NT, D], f32, tag=f"tab_{name}")
            nc.sync.dma_start(out=t, in_=ap)
            tab_sb[name] = t

        def load_and_ssq(pr):
            """Load q,k; return raws and rsqrt(sumsq) [128, 32] (q|k) via
            DVE Newton (no activation-table traffic)."""
            raws = {}
            ssq2 = small.tile([128, 2 * NT], f32, tag="ssq2")
            for col, (which, view) in enumerate((("q", qv), ("k", kv))):
                raw = ld.tile([128, NT, D], f32, tag=f"raw_{which}")
                nc.sync.dma_start(out=raw, in_=view[pr])
                raws[which] = raw
                sq = prep.tile([128, NT, D], f32, tag="sq")
                nc.vector.tensor_mul(sq, raw, raw)
                nc.vector.tensor_reduce(
                    out=ssq2[:, col * NT:(col + 1) * NT], in_=sq,
                    axis=mybir.AxisListType.X, op=mybir.AluOpType.add)
            # Newton rsqrt on GpSimd: keeps the 20-op serial chain off the
            # DVE FIFO (it was stretching pair-0 prep latency badly)
            y = small.tile([128, 2 * NT], f32, tag="ny")
            nc.vector.memset(y, 0.12)
            t = small.tile([128, 2 * NT], f32, tag="nt")
            for _ in range(5):  # y <- y*(1.5 - 0.5*x*y^2)
                nc.gpsimd.tensor_tensor(t, y, y, mybir.AluOpType.mult)
                nc.gpsimd.tensor_tensor(t, t, ssq2, mybir.AluOpType.mult)
                nc.gpsimd.tensor_scalar(out=t, in0=t, scalar1=-0.5,
                                        scalar2=1.5,
                                        op0=mybir.AluOpType.mult,
                                        op1=mybir.AluOpType.add)
                nc.gpsimd.tensor_tensor(y, y, t, mybir.AluOpType.mult)
            return raws, y

        def rope_norm(raw, rinv, cos_sb, sin_sb, tag):
            """bf16 rope(raw)*rinv; rinv is a [128, NT] slice."""
            t1 = prep.tile([128, NT, D], f32, tag="t1")
            nc.vector.tensor_mul(t1, raw, cos_sb)
            t2 = prep.tile([128, NT, D], f32, tag="t2")
            nc.vector.tensor_mul(t2[:, :, 0:half], raw[:, :, half:D],
                                 sin_sb[:, :, 0:half])
            nc.vector.tensor_mul(t2[:, :, half:D], raw[:, :, 0:half],
                                 sin_sb[:, :, half:D])
            rope = prep.tile([128, NT, D], f32, tag="rope")
            nc.vector.tensor_add(rope, t1, t2)
            rb = bass.AP(tensor=rinv.tensor, offset=rinv.offset,
                         ap=[*rinv.ap, [0, D]])
            out = prep.tile([128, NT, D], bf16, tag=f"n_{tag}")
            nc.vector.tensor_mul(out, rope, rb)
            return out

        def transpose_to(dst_T, src):
            # src [128, NT, 64] bf16 -> dst_T [64, N] bf16 via PE transposes
            for g in range(NT // 4):
                ps = tpsum.tile([64, 4, 128], bf16, tag="tp")
                for u in range(4):
                    t = 4 * g + u
                    nc.tensor.transpose(out=ps[:, u, :], in_=src[:, t, :],
                                        identity=identity_bf)
                nc.vector.tensor_copy(
                    out=dst_T[:, g * 512:(g + 1) * 512].rearrange(
                        "p (a b) -> p a b", a=4),
                    in_=ps)

        def do_prep(pr):
            raws, rinv2 = load_and_ssq(pr)
            qn = rope_norm(raws["q"], rinv2[:, 0:NT],
                           tab_sb["qcos"], tab_sb["qsin"], "q")
            qT = pairp.tile([128, N], bf16, tag="qT")
            transpose_to(qT[0:64, :], qn)
            # duplicate into partitions 64:127 so row-packed matmuls
            # (tile_position row 64) can read lhsT/rhs from the upper half
            nc.sync.dma_start(out=qT[64:128, :], in_=qT[0:64, :])

            kn = rope_norm(raws["k"], rinv2[:, NT:2 * NT],
                           tab_sb["kcos"], tab_sb["ksin"], "k")
            kT = pairp.tile([128, N], bf16, tag="kT")
            transpose_to(kT[0:64, :], kn)
            nc.sync.dma_start(out=kT[64:128, :], in_=kT[0:64, :])

            vext = pairp.tile([128, NT, D + 1], bf16, tag="vext")
            nc.gpsimd.dma_start(out=vext[:, :, 0:D], in_=vv[pr])  # f32->bf16
            nc.vector.memset(vext[:, :, D:D + 1], 1.0)
            return qT, kT, vext

        def do_main(pr, qT, kT, vext):
            for ic in range(IC):
                esb = epool.tile([128, NT, ICW], bf16, tag="E")
                op = opsum.tile([D + 1, ICW], f32, tag="O")
                for g in range(NT // 2):
                    sp = spsum.tile([128, 2, ICW], f32, tag="S")
                    for u in range(2):
                        jt = 2 * g + u
                        lo = 64 * u
                        nc.tensor.matmul(
                            out=sp[:, u, :],
                            lhsT=kT[lo:lo + 64, jt * 128:(jt + 1) * 128],
                            rhs=qT[lo:lo + 64, ic * ICW:(ic + 1) * ICW],
                            start=True, stop=True,
                            tile_position=(lo, 0))
                    nc.scalar.activation(out=esb[:, 2 * g:2 * g + 2, :], in_=sp,
                                         func=AF.Exp, scale=float(D ** -0.5),
                                         bias=zbias)
                    for u in range(2):
                        jt = 2 * g + u
                        nc.tensor.matmul(out=op, lhsT=vext[:, jt, :],
                                         rhs=esb[:, jt, :],
                                         start=(jt == 0), stop=(jt == NT - 1))
                oc = opool.tile([D + 1, ICW], f32, tag="oc")
                nc.vector.tensor_copy(out=oc, in_=op)
                ot = tpsum.tile([128, 4, D + 1], f32, tag="tp")
                for tt in range(4):
                    nc.tensor.transpose(out=ot[:, tt, :],
                                        in_=oc[:, tt * 128:(tt + 1) * 128],
                                        identity=identity[0:D + 1, 0:D + 1])
                ro = small.tile([128, 4], f32, tag="ro")
                nc.vector.reciprocal(out=ro, in_=ot[:, :, D])
                osb = opool.tile([128, 4, D], f32, tag="osb")
                rob = bass.AP(tensor=ro.tensor, offset=ro.offset,
                              ap=[*ro.ap, [0, D]])
                nc.vector.tensor_mul(osb, ot[:, :, 0:D], rob)
                nc.sync.dma_start(out=outv[pr, ic], in_=osb)

        # software pipeline: pair p+1's prep is emitted (priority-ordered)
        # ahead of pair p's main loop so DVE prep overlaps PE/ACT main work
        handles = do_prep(0)
        for pr in range(PPC):
            nxt = do_prep(pr + 1) if pr + 1 < PPC else None
            do_main(pr, *handles)
            handles = nxt

    nc.compile()
    _CACHE["nc"] = nc
    return nc


def kernel(q, k, v, q_scale, k_scale):
    global LAST_RESULTS
    from concourse.bass_utils import run_bass_kernel_spmd

    nc = _build()
    q = np.ascontiguousarray(np.asarray(q, dtype=np.float32))
    k = np.ascontiguousarray(np.asarray(k, dtype=np.float32))
    v = np.ascontiguousarray(np.asarray(v, dtype=np.float32))
    qcos, qsin, kcos, ksin = _rope_tables(np.asarray(q_scale),
                                          np.asarray(k_scale))

    # pair index = b*H + h; core c owns pairs [c*PPC, (c+1)*PPC)
    qp = q.reshape(PAIRS, N, D)
    kp = k.reshape(PAIRS, N, D)
    vp = v.reshape(PAIRS, N, D)
    in_maps = []
    for c in range(N_CORES):
        sl = slice(c * PPC, (c + 1) * PPC)
        in_maps.append({
            "q4": qp[sl], "k4": kp[sl], "v4": vp[sl],
            "qcos": qcos, "qsin": qsin, "kcos": kcos, "ksin": ksin,
        })

    trace = bool(int(os.environ.get("KERNEL_TRACE", "0")))
    kwargs = {}
    if trace and os.environ.get("KERNEL_TRACE_DIR"):
        kwargs["tmpdir"] = os.environ["KERNEL_TRACE_DIR"]
    res = run_bass_kernel_spmd(nc, in_maps, list(range(N_CORES)),
                               trace=trace, **kwargs)
    LAST_RESULTS = res

    outp = np.concatenate([res.results[c]["out4"] for c in range(N_CORES)],
                          axis=0)                       # [32, N, D]
    out = outp.reshape(B, H, N, D).transpose(0, 2, 1, 3).reshape(B, N, H * D)
    return np.ascontiguousarray(out)
